# revision 1
# baseline (speedup 1.0000x reference)
"""Trainium2 Bass kernel for nn_ContrastiveLoss (SimCLR-style, N=8192, D=128).

Sharding: rows of the NxN sim matrix split across 8 cores (1024 rows each).
Each core receives the full z = concat(emb0, emb1) ROTATED so its own rows
come first (np.roll(z, -core*1024, axis=0)).  With that rotation the diagonal
of row-block b sits at local columns [b*128, b*128+128) and the positive pair
at local columns [4096+b*128, ...), identical on every core -> one SPMD
program, no collectives.

Math (per row r, fixed max = 1.0 since cosine sim <= 1):
  e_j  = exp(10*G_rj - 10),  S_r = sum_j e_j - e_rr
  loss_r = lse_r - 10*G_pos = (10 + ln S_r) - (ln e_pos + 10) = ln S_r - ln e_pos
  loss   = mean_r(loss_r);  per-core output = [128,1] partial sums of loss_r.

Engine split per core: PE does z_blk @ z^T (bf16 operands, fp32 psum)
plus the zn transposes; ACT does exp(10x-10) on each [128,2048] psum chunk
with accum_out row-sums; DVE does norms, psum->bf16 casts (batched 512 wide)
and diag/pos extraction from the exp output in SBUF.
"""

import sys

sys.path.insert(0, "/opt/trn_rl_repo")

from contextlib import ExitStack

import numpy as np

import concourse.bass as bass
import concourse.bacc as bacc
import concourse.tile as tile
from concourse import mybir
from concourse import bass_utils
from concourse.masks import make_identity

B = 4096
D = 128
N = 2 * B            # 8192 rows of z
NCORES = 8
ROWS = N // NCORES   # 1024 rows per core
NBLK = ROWS // 128   # 8 row-blocks per core
CHUNK = 2048         # psum tile width (4 banks)
NCHUNK = N // CHUNK  # 4 column chunks
SEG = 512            # matmul moving-operand width
NTILE = N // 128     # 64 partition-tiles of z
GRP = 8              # tiles per DMA / norm group
INV_T = 10.0         # 1/temperature
EPS = 1e-8

F32 = mybir.dt.float32
BF16 = mybir.dt.bfloat16
AX = mybir.AxisListType
AF = mybir.ActivationFunctionType


def _build() -> bass.Bass:
    nc = bacc.Bacc(None)
    z_in = nc.declare_dram_parameter("z", [N, D], F32, isOutput=False)
    out = nc.declare_dram_parameter("partial", [128, 1], F32, isOutput=True)

    z_re = z_in.rearrange("(n p) d -> p n d", p=128)  # row = n*128 + p

    with tile.TileContext(nc) as tc:
        with ExitStack() as ctx:
            persist = ctx.enter_context(tc.tile_pool(name="persist", bufs=1))
            work = ctx.enter_context(tc.tile_pool(name="work", bufs=3))
            junkp = ctx.enter_context(tc.tile_pool(name="junk", bufs=3))
            psum = ctx.enter_context(tc.tile_pool(name="psum", bufs=2, space="PSUM"))

            ident = persist.tile([128, 128], BF16)
            make_identity(nc, ident)
            # non-Copy activations need bias as an SBUF AP
            b_zero = persist.tile([128, 1], F32)
            nc.vector.memset(b_zero, 0.0)
            b_neg10 = persist.tile([128, 1], F32)
            nc.vector.memset(b_neg10, -INV_T)

            # ---- load z + per-group norms + normalize + transpose --------
            # Per 8-tile group: DMA -> sumsq -> rsqrt-norm -> bf16 zn ->
            # 8 PE transposes into one psum bank -> one 1024-wide cast.
            z_sb = persist.tile([128, NTILE, D], F32)
            sq = persist.tile([128, NTILE, D], F32)
            rn = persist.tile([128, NTILE], F32)
            zn_all = persist.tile([128, NTILE, D], BF16)
            znT = [
                persist.tile([128, CHUNK], BF16, tag=f"znT{j}", name=f"znT{j}")
                for j in range(NCHUNK)
            ]
            acc = persist.tile([128, NBLK, NCHUNK], F32)   # per-chunk exp sums
            e_diag = persist.tile([128, NBLK], F32)
            e_pos = persist.tile([128, NBLK], F32)

            # all input DMAs up front; the sync queue streams them back-to-back
            for i in range(NTILE // GRP):
                sl = slice(i * GRP, (i + 1) * GRP)
                nc.sync.dma_start(out=z_sb[:, sl, :], in_=z_re[:, sl, :])

            def norm_group(i):
                sl = slice(i * GRP, (i + 1) * GRP)
                nc.vector.tensor_mul(sq[:, sl, :], z_sb[:, sl, :], z_sb[:, sl, :])
                nc.vector.reduce_sum(rn[:, sl], sq[:, sl, :], axis=AX.X)
                nc.scalar.activation(rn[:, sl], rn[:, sl], AF.Sqrt, bias=b_zero)
                nc.vector.tensor_scalar_max(rn[:, sl], rn[:, sl], EPS)
                nc.vector.reciprocal(rn[:, sl], rn[:, sl])
                nc.vector.tensor_mul(
                    zn_all[:, sl, :],
                    z_sb[:, sl, :],
                    rn[:, sl].broadcast_to((128, GRP, D)),
                )
                tp = psum.tile([128, GRP * 128], BF16, tag="pp", name="tp")
                for q in range(GRP):
                    nc.tensor.transpose(
                        tp[:, q * 128 : (q + 1) * 128],
                        zn_all[:, i * GRP + q, :],
                        ident,
                    )
                j, k = divmod(i * GRP * 128, CHUNK)
                nc.vector.tensor_copy(znT[j][:, k : k + GRP * 128], tp)

            def emit_block(b, c):
                lhsT = znT[0][:, b * 128 : (b + 1) * 128]  # block cols < 1024
                pt = psum.tile([128, CHUNK], F32, tag="pp", name="pt")
                for s in range(CHUNK // SEG):
                    nc.tensor.matmul(
                        pt[:, s * SEG : (s + 1) * SEG],
                        lhsT,
                        znT[c][:, s * SEG : (s + 1) * SEG],
                        start=True,
                        stop=True,
                    )
                ej = junkp.tile([128, CHUNK], F32, tag="ej", name="ej")
                nc.scalar.activation(
                    ej, pt, AF.Exp, scale=INV_T, bias=b_neg10,
                    accum_out=acc[:, b, c : c + 1],
                )
                if c == 0:  # e_rr at cols b*128..+128 of chunk 0
                    scr = work.tile([128, 128], F32, tag="scr", name="scr")
                    nc.vector.tensor_mul(scr, ej[:, b * 128 : b * 128 + 128], ident)
                    nc.vector.reduce_sum(e_diag[:, b : b + 1], scr, axis=AX.X)
                if c == 2:  # e_pos at cols 4096 + b*128..+128
                    scr2 = work.tile([128, 128], F32, tag="scr2", name="scr2")
                    nc.vector.tensor_mul(scr2, ej[:, b * 128 : b * 128 + 128], ident)
                    nc.vector.reduce_sum(e_pos[:, b : b + 1], scr2, axis=AX.X)

            # Pass 0 interleaves the remaining norm groups PAIRWISE so the
            # 2-slot psum round-robin keeps consecutive pt tiles on opposite
            # slots (tp pairs between pt pairs); all znT chunks are ready
            # before pass 1 and the exp stream starts as soon as znT[0] is.
            norm_group(0); norm_group(1)
            emit_block(0, 0); emit_block(1, 0)
            norm_group(2); norm_group(3)
            emit_block(2, 0); emit_block(3, 0)
            norm_group(4); norm_group(5)
            emit_block(4, 0); emit_block(5, 0)
            norm_group(6); norm_group(7)
            emit_block(6, 0); emit_block(7, 0)
            for c in range(1, NCHUNK):
                for b in range(NBLK):
                    emit_block(b, c)

            # ---- epilogue ------------------------------------------------
            sumexp = persist.tile([128, NBLK], F32)
            nc.vector.reduce_sum(sumexp, acc, axis=AX.X)      # [128,8,4] -> [128,8]
            S = persist.tile([128, NBLK], F32)
            nc.vector.tensor_sub(S, sumexp, e_diag)
            lnS = persist.tile([128, NBLK], F32)
            nc.scalar.activation(lnS, S, AF.Ln, bias=b_zero)
            lnp = persist.tile([128, NBLK], F32)
            nc.scalar.activation(lnp, e_pos, AF.Ln, bias=b_zero)
            contrib = persist.tile([128, NBLK], F32)
            nc.vector.tensor_sub(contrib, lnS, lnp)
            total = persist.tile([128, 1], F32)
            nc.vector.reduce_sum(total, contrib, axis=AX.X)
            nc.sync.dma_start(out=out[:, :], in_=total)

    nc.compile()
    return nc


_NC = None


def _get_nc() -> bass.Bass:
    global _NC
    if _NC is None:
        _NC = _build()
    return _NC


def kernel(emb0: np.ndarray, emb1: np.ndarray) -> np.ndarray:
    z = np.concatenate(
        [np.asarray(emb0, np.float32), np.asarray(emb1, np.float32)], axis=0
    )
    in_maps = [
        {"z": np.ascontiguousarray(np.roll(z, -c * ROWS, axis=0))}
        for c in range(NCORES)
    ]
    res = bass_utils.run_bass_kernel_spmd(_get_nc(), in_maps, core_ids=list(range(NCORES)))
    total = sum(float(r["partial"].sum(dtype=np.float64)) for r in res.results)
    return np.asarray(np.float32(total / N))



# revision 5
# speedup vs baseline: 1.0169x; 1.0169x over previous
"""Trainium2 Bass kernel for nn_ContrastiveLoss (SimCLR-style, N=8192, D=128).

Sharding: rows of the NxN sim matrix split across 8 cores (1024 rows each).
Each core receives the full z = concat(emb0, emb1) ROTATED so its own rows
come first (np.roll(z, -core*1024, axis=0)).  With that rotation the positive
pair of local row l is local row l+4096 on every core -> one SPMD program,
no collectives.

Math (per row r, fixed max = 1.0 since cosine sim <= 1):
  S_r    = sum_j exp(10*G_rj - 10) - exp(10*G_rr - 10)   [G_rr = 1 => subtract 1]
  loss_r = (10 + ln S_r) - 10*G_pos
  loss   = mean_r(loss_r);  per-core output = [128,1] partial sums of loss_r.

v2 engine split (ACT is the bottleneck; everything else hides under it):
  ACT : one natural_log_exp table load, 32x exp([128,2048] psum) with
        accum_out row sums, one tiny Ln at the end.  No sqrt (-> no table
        thrash), no diag/pos extraction feeding back into the stream.
  PE  : only the 64 G-matmuls (bf16, 1024-wide moving operand).  No
        transposes (moved to the DMA xbar).
  DVE : norms via mul+reduce + Newton rsqrt (fast-inverse-sqrt bit trick,
        ~5e-6 rel err), normalize to bf16, positive-pair dot products,
        epilogue.
  DMA : input load + zn -> znT transposes via dma_start_transpose (xbar),
        one 16-tile group per instruction.
"""

import sys

sys.path.insert(0, "/opt/trn_rl_repo")

from contextlib import ExitStack

import numpy as np

import concourse.bass as bass
import concourse.bacc as bacc
import concourse.tile as tile
from concourse import mybir
from concourse import bass_utils

B = 4096
D = 128
N = 2 * B            # 8192 rows of z
NCORES = 8
ROWS = N // NCORES   # 1024 rows per core
NBLK = ROWS // 128   # 8 row-blocks per core
CHUNK = 2048         # psum tile width (4 banks)
NCHUNK = N // CHUNK  # 4 column chunks
SEG = 512            # matmul moving-operand width
NTILE = N // 128     # 64 partition-tiles of z
GRP = 16             # tiles per DMA / norm group
NGRP = NTILE // GRP  # 4 groups
INV_T = 10.0         # 1/temperature
MAGIC = 0x5F3759DF

F32 = mybir.dt.float32
BF16 = mybir.dt.bfloat16
U32 = mybir.dt.uint32
AX = mybir.AxisListType
AF = mybir.ActivationFunctionType
ALU = mybir.AluOpType


def _build() -> bass.Bass:
    nc = bacc.Bacc(None)
    z_in = nc.declare_dram_parameter("z", [N, D], F32, isOutput=False)
    out = nc.declare_dram_parameter("partial", [128, 1], F32, isOutput=True)

    z_re = z_in.rearrange("(n p) d -> p n d", p=128)  # row = n*128 + p

    with tile.TileContext(nc) as tc:
        with ExitStack() as ctx:
            persist = ctx.enter_context(tc.tile_pool(name="persist", bufs=1))
            junkp = ctx.enter_context(tc.tile_pool(name="junk", bufs=2))
            psum = ctx.enter_context(tc.tile_pool(name="psum", bufs=2, space="PSUM"))

            # non-Copy activations need bias as an SBUF AP
            b_zero = persist.tile([128, 1], F32)
            nc.vector.memset(b_zero, 0.0)
            b_neg10 = persist.tile([128, 1], F32)
            nc.vector.memset(b_neg10, -INV_T)
            magic = persist.tile([128, 1], U32)
            nc.vector.memset(magic, MAGIC)

            z_sb = persist.tile([128, NTILE, D], F32)
            sq = persist.tile([128, NTILE, D], F32)
            ss = persist.tile([128, NTILE], F32)       # sumsq -> clamped
            rn = persist.tile([128, NTILE], F32)       # 1/norm (Newton rsqrt)
            nt0 = persist.tile([128, NTILE], F32)      # newton temps
            nt1 = persist.tile([128, NTILE], F32)
            zn_all = persist.tile([128, NTILE, D], BF16)
            znT = persist.tile([128, NTILE, D], BF16)  # [d, tile, row%128]
            acc = persist.tile([128, NBLK, NCHUNK], F32)   # per-chunk exp sums
            pprod = persist.tile([128, NBLK, D], F32)      # pos-pair products
            gpos = persist.tile([128, NBLK], F32)          # G_pos per row

            def load_group(g):
                sl = slice(g * GRP, (g + 1) * GRP)
                nc.sync.dma_start(out=z_sb[:, sl, :], in_=z_re[:, sl, :])

            def norm_group(g):
                sl = slice(g * GRP, (g + 1) * GRP)
                nc.vector.tensor_mul(sq[:, sl, :], z_sb[:, sl, :], z_sb[:, sl, :])
                nc.vector.reduce_sum(ss[:, sl], sq[:, sl, :], axis=AX.X)
                nc.vector.tensor_scalar_max(ss[:, sl], ss[:, sl], 1e-16)
                # y0 = bitcast(MAGIC - (bits(s) >> 1)); two Newton steps
                # (DVE int add saturates, so use tensor_sub from a const tile)
                s_u = ss[:, sl].bitcast(U32)
                y_u = rn[:, sl].bitcast(U32)
                t_u = nt0[:, sl].bitcast(U32)
                nc.vector.tensor_scalar(t_u, s_u, 1, None, ALU.logical_shift_right)
                nc.vector.tensor_sub(y_u, magic.broadcast_to((128, GRP)), t_u)
                for _ in range(2):
                    nc.vector.tensor_mul(nt0[:, sl], rn[:, sl], rn[:, sl])
                    nc.vector.tensor_mul(nt1[:, sl], nt0[:, sl], ss[:, sl])
                    nc.vector.tensor_scalar(
                        nt0[:, sl], nt1[:, sl], -0.5, 1.5, ALU.mult, ALU.add
                    )
                    nc.vector.tensor_mul(rn[:, sl], rn[:, sl], nt0[:, sl])
                nc.vector.tensor_mul(
                    zn_all[:, sl, :],
                    z_sb[:, sl, :],
                    rn[:, sl].broadcast_to((128, GRP, D)),
                )

            def transpose_group(g):
                sl = slice(g * GRP, (g + 1) * GRP)
                nc.sync.dma_start_transpose(znT[:, sl, :], zn_all[:, sl, :])

            def emit_chunk(b, c):
                lhsT = znT[:, b, :]  # [128(d), 128 rows] : block cols b*128..
                pt = psum.tile([128, CHUNK], F32, tag="pp", name="pt")
                for s in range(CHUNK // SEG):
                    cs = c * (CHUNK // D) + s * (SEG // D)
                    nc.tensor.matmul(
                        pt[:, s * SEG : (s + 1) * SEG],
                        lhsT,
                        znT[:, cs : cs + SEG // D, :],
                        start=True,
                        stop=True,
                    )
                ej = junkp.tile([128, CHUNK], BF16, tag="ej", name="ej")
                nc.scalar.activation(
                    ej, pt, AF.Exp, scale=INV_T, bias=b_neg10,
                    accum_out=acc[:, b, c : c + 1],
                )

            # ---- prologue: interleave DMA loads, norm chains, transposes ----
            load_group(0)
            load_group(1)
            norm_group(0)
            transpose_group(0)
            load_group(2)
            norm_group(1)
            transpose_group(1)
            load_group(3)
            norm_group(2)
            transpose_group(2)
            # positive-pair dots: G_pos[p, b] = sum_d zn[p,b,d]*zn[p,b+32,d]
            nc.vector.tensor_mul(pprod, zn_all[:, 0:NBLK, :], zn_all[:, 32 : 32 + NBLK, :])
            nc.vector.reduce_sum(gpos, pprod, axis=AX.X)
            norm_group(3)
            transpose_group(3)

            # ---- main exp stream: c-outer so znT groups arrive in time ----
            for c in range(NCHUNK):
                for b in range(NBLK):
                    emit_chunk(b, c)

            # ---- epilogue ------------------------------------------------
            sumexp = persist.tile([128, NBLK], F32)
            nc.vector.reduce_sum(sumexp, acc, axis=AX.X)      # [128,8,4] -> [128,8]
            S = persist.tile([128, NBLK], F32)
            nc.vector.tensor_scalar_add(S, sumexp, -1.0)      # drop diagonal (=1)
            lnS = persist.tile([128, NBLK], F32)
            nc.scalar.activation(lnS, S, AF.Ln, bias=b_zero)
            pterm = persist.tile([128, NBLK], F32)
            nc.vector.tensor_scalar(
                pterm, gpos, -INV_T, INV_T, ALU.mult, ALU.add  # 10 - 10*G_pos
            )
            contrib = persist.tile([128, NBLK], F32)
            nc.vector.tensor_add(contrib, lnS, pterm)
            total = persist.tile([128, 1], F32)
            nc.vector.reduce_sum(total, contrib, axis=AX.X)
            nc.sync.dma_start(out=out[:, :], in_=total)

    nc.compile()
    return nc


_NC = None


def _get_nc() -> bass.Bass:
    global _NC
    if _NC is None:
        _NC = _build()
    return _NC


def kernel(emb0: np.ndarray, emb1: np.ndarray) -> np.ndarray:
    z = np.concatenate(
        [np.asarray(emb0, np.float32), np.asarray(emb1, np.float32)], axis=0
    )
    in_maps = [
        {"z": np.ascontiguousarray(np.roll(z, -c * ROWS, axis=0))}
        for c in range(NCORES)
    ]
    res = bass_utils.run_bass_kernel_spmd(_get_nc(), in_maps, core_ids=list(range(NCORES)))
    total = sum(float(r["partial"].sum(dtype=np.float64)) for r in res.results)
    return np.asarray(np.float32(total / N))


# revision 6
# speedup vs baseline: 1.0253x; 1.0083x over previous
"""Trainium2 Bass kernel for nn_ContrastiveLoss (SimCLR-style, N=8192, D=128).

Sharding: rows of the NxN sim matrix split across 8 cores (1024 rows each).
Each core receives the full z = concat(emb0, emb1) ROTATED so its own rows
come first (np.roll(z, -core*1024, axis=0)).  With that rotation the positive
pair of local row l is local row l+4096 on every core -> one SPMD program,
no collectives.

Math (per row r, fixed max = 1.0 since cosine sim <= 1):
  S_r    = sum_j exp(10*G_rj - 10) - exp(10*G_rr - 10)   [G_rr = 1 => subtract 1]
  loss_r = (10 + ln S_r) - 10*G_pos
  loss   = mean_r(loss_r);  per-core output = [128,1] partial sums of loss_r.

v2 engine split (ACT is the bottleneck; everything else hides under it):
  ACT : one natural_log_exp table load, 32x exp([128,2048] psum) with
        accum_out row sums, one tiny Ln at the end.  No sqrt (-> no table
        thrash), no diag/pos extraction feeding back into the stream.
  PE  : only the 64 G-matmuls (bf16, 1024-wide moving operand).  No
        transposes (moved to the DMA xbar).
  DVE : norms via mul+reduce + Newton rsqrt (fast-inverse-sqrt bit trick,
        ~5e-6 rel err), normalize to bf16, positive-pair dot products,
        epilogue.
  DMA : input load + zn -> znT transposes via dma_start_transpose (xbar),
        one 16-tile group per instruction.
"""

import sys

sys.path.insert(0, "/opt/trn_rl_repo")

from contextlib import ExitStack

import numpy as np

import concourse.bass as bass
import concourse.bacc as bacc
import concourse.tile as tile
from concourse import mybir
from concourse import bass_utils

B = 4096
D = 128
N = 2 * B            # 8192 rows of z
NCORES = 8
ROWS = N // NCORES   # 1024 rows per core
NBLK = ROWS // 128   # 8 row-blocks per core
CHUNK = 2048         # psum tile width (4 banks)
NCHUNK = N // CHUNK  # 4 column chunks
SEG = 512            # matmul moving-operand width
NTILE = N // 128     # 64 partition-tiles of z
GRP = 16             # tiles per DMA / norm group
NGRP = NTILE // GRP  # 4 groups
INV_T = 10.0         # 1/temperature
MAGIC = 0x5F3759DF

F32 = mybir.dt.float32
BF16 = mybir.dt.bfloat16
U32 = mybir.dt.uint32
AX = mybir.AxisListType
AF = mybir.ActivationFunctionType
ALU = mybir.AluOpType


def _build() -> bass.Bass:
    nc = bacc.Bacc(None)
    z_in = nc.declare_dram_parameter("z", [N, D], F32, isOutput=False)
    out = nc.declare_dram_parameter("partial", [128, 1], F32, isOutput=True)

    z_re = z_in.rearrange("(n p) d -> p n d", p=128)  # row = n*128 + p

    with tile.TileContext(nc) as tc:
        with ExitStack() as ctx:
            persist = ctx.enter_context(tc.tile_pool(name="persist", bufs=1))
            junkp = ctx.enter_context(tc.tile_pool(name="junk", bufs=2))
            psum = ctx.enter_context(tc.tile_pool(name="psum", bufs=2, space="PSUM"))

            # non-Copy activations need bias as an SBUF AP
            b_zero = persist.tile([128, 1], F32)
            nc.vector.memset(b_zero, 0.0)
            b_neg10 = persist.tile([128, 1], F32)
            nc.vector.memset(b_neg10, -INV_T)
            magic = persist.tile([128, 1], U32)
            nc.vector.memset(magic, MAGIC)

            z_sb = persist.tile([128, NTILE, D], F32)
            sq = persist.tile([128, NTILE, D], F32)
            ss = persist.tile([128, NTILE], F32)       # sumsq -> clamped
            rn = persist.tile([128, NTILE], F32)       # 1/norm (Newton rsqrt)
            nt0 = persist.tile([128, NTILE], F32)      # newton temps
            nt1 = persist.tile([128, NTILE], F32)
            zn_all = persist.tile([128, NTILE, D], BF16)
            znT = persist.tile([128, NTILE, D], BF16)  # [d, tile, row%128]
            acc = persist.tile([128, NBLK, NCHUNK], F32)   # per-chunk exp sums
            pprod = persist.tile([128, NBLK, D], F32)      # pos-pair products
            gpos = persist.tile([128, NBLK], F32)          # G_pos per row

            def load_group(g):
                sl = slice(g * GRP, (g + 1) * GRP)
                nc.sync.dma_start(out=z_sb[:, sl, :], in_=z_re[:, sl, :])

            def norm_group(g):
                sl = slice(g * GRP, (g + 1) * GRP)
                nc.vector.tensor_mul(sq[:, sl, :], z_sb[:, sl, :], z_sb[:, sl, :])
                nc.vector.reduce_sum(ss[:, sl], sq[:, sl, :], axis=AX.X)
                nc.vector.tensor_scalar_max(ss[:, sl], ss[:, sl], 1e-16)
                # y0 = bitcast(MAGIC - (bits(s) >> 1)); two Newton steps
                # (DVE int add saturates, so use tensor_sub from a const tile)
                s_u = ss[:, sl].bitcast(U32)
                y_u = rn[:, sl].bitcast(U32)
                t_u = nt0[:, sl].bitcast(U32)
                nc.vector.tensor_scalar(t_u, s_u, 1, None, ALU.logical_shift_right)
                nc.vector.tensor_sub(y_u, magic.broadcast_to((128, GRP)), t_u)
                for _ in range(2):
                    nc.vector.tensor_mul(nt0[:, sl], rn[:, sl], rn[:, sl])
                    nc.vector.tensor_mul(nt1[:, sl], nt0[:, sl], ss[:, sl])
                    nc.vector.tensor_scalar(
                        nt0[:, sl], nt1[:, sl], -0.5, 1.5, ALU.mult, ALU.add
                    )
                    nc.vector.tensor_mul(rn[:, sl], rn[:, sl], nt0[:, sl])
                nc.vector.tensor_mul(
                    zn_all[:, sl, :],
                    z_sb[:, sl, :],
                    rn[:, sl].broadcast_to((128, GRP, D)),
                )

            def transpose_group(g):
                sl = slice(g * GRP, (g + 1) * GRP)
                nc.sync.dma_start_transpose(znT[:, sl, :], zn_all[:, sl, :])

            def emit_chunk(b, c):
                lhsT = znT[:, b, :]  # [128(d), 128 rows] : block cols b*128..
                pt = psum.tile([128, CHUNK], F32, tag="pp", name="pt")
                for s in range(CHUNK // SEG):
                    cs = c * (CHUNK // D) + s * (SEG // D)
                    nc.tensor.matmul(
                        pt[:, s * SEG : (s + 1) * SEG],
                        lhsT,
                        znT[:, cs : cs + SEG // D, :],
                        start=True,
                        stop=True,
                    )
                ej = junkp.tile([128, CHUNK], BF16, tag="ej", name="ej")
                nc.scalar.activation(
                    ej, pt, AF.Exp, scale=INV_T, bias=b_neg10,
                    accum_out=acc[:, b, c : c + 1],
                )

            # ---- prologue: interleave DMA loads, norm chains, transposes ----
            # tile_wait_until holds non-critical groups back so the greedy
            # scheduler doesn't interleave their big DVE ops into group 0's
            # critical chain (which gates the first exp).
            load_group(0)
            norm_group(0)
            transpose_group(0)
            with tc.tile_wait_until(0.003):
                load_group(1)
            with tc.tile_wait_until(0.005):
                load_group(2)
            with tc.tile_wait_until(0.007):
                load_group(3)
            with tc.tile_wait_until(0.009):
                norm_group(1)
                transpose_group(1)
            with tc.tile_wait_until(0.015):
                norm_group(2)
                transpose_group(2)
            with tc.tile_wait_until(0.021):
                norm_group(3)
                transpose_group(3)
            with tc.tile_wait_until(0.027):
                # positive-pair dots: G_pos[p,b] = sum_d zn[p,b,d]*zn[p,b+32,d]
                nc.vector.tensor_mul(
                    pprod, zn_all[:, 0:NBLK, :], zn_all[:, 32 : 32 + NBLK, :]
                )
                nc.vector.reduce_sum(gpos, pprod, axis=AX.X)

            # ---- main exp stream: c-outer so znT groups arrive in time ----
            for c in range(NCHUNK):
                for b in range(NBLK):
                    emit_chunk(b, c)

            # ---- epilogue ------------------------------------------------
            sumexp = persist.tile([128, NBLK], F32)
            nc.vector.reduce_sum(sumexp, acc, axis=AX.X)      # [128,8,4] -> [128,8]
            S = persist.tile([128, NBLK], F32)
            nc.vector.tensor_scalar_add(S, sumexp, -1.0)      # drop diagonal (=1)
            lnS = persist.tile([128, NBLK], F32)
            nc.scalar.activation(lnS, S, AF.Ln, bias=b_zero)
            pterm = persist.tile([128, NBLK], F32)
            nc.vector.tensor_scalar(
                pterm, gpos, -INV_T, INV_T, ALU.mult, ALU.add  # 10 - 10*G_pos
            )
            contrib = persist.tile([128, NBLK], F32)
            nc.vector.tensor_add(contrib, lnS, pterm)
            total = persist.tile([128, 1], F32)
            nc.vector.reduce_sum(total, contrib, axis=AX.X)
            nc.sync.dma_start(out=out[:, :], in_=total)

    nc.compile()
    return nc


_NC = None


def _get_nc() -> bass.Bass:
    global _NC
    if _NC is None:
        _NC = _build()
    return _NC


def kernel(emb0: np.ndarray, emb1: np.ndarray) -> np.ndarray:
    z = np.concatenate(
        [np.asarray(emb0, np.float32), np.asarray(emb1, np.float32)], axis=0
    )
    in_maps = [
        {"z": np.ascontiguousarray(np.roll(z, -c * ROWS, axis=0))}
        for c in range(NCORES)
    ]
    res = bass_utils.run_bass_kernel_spmd(_get_nc(), in_maps, core_ids=list(range(NCORES)))
    total = sum(float(r["partial"].sum(dtype=np.float64)) for r in res.results)
    return np.asarray(np.float32(total / N))
